# revision 35
# baseline (speedup 1.0000x reference)
"""Trainium2 Bass kernel for windowed multi-head attention with relative
position bias (Swin-style block):

    qkv = x @ qkv_w.T + [q_bias, 0, v_bias]
    q, k, v = split(qkv);  q *= hd**-0.5
    attn = softmax(q @ k.T + rel_table[rel_index])
    out  = (attn @ v) @ proj_w.T + proj_b

Shapes: x [8, 32, 32, 768], 12 heads, head_dim 64, N=1024 tokens.

Sharding: pure data-parallel — one batch element per NeuronCore, 8 cores,
no collectives. Each core runs an identical NEFF on its own slice.

Per-core dataflow (fp16 operands, fp32 PSUM accumulate):
  xT [c,t] as lhsT, wv        -> V natural [t,o'] + ones col (v_aug)
  wqk pair-slices as lhsT, xT -> Q^T packed per pair / K^T zero-padded
    per head (full-128-row stationaries keep fast-weight-load enabled)
  S^T[k,q] = K_pad(h)^T @ Q  (per head, per k-tile; fp16)
  P^T = exp(S^T) (ACT) * expB (host-precomputed fp16 bias, DVE/Pool)
  O[q,d]+sums = pt-chunk^T @ [V_h|1]  -- natural orientation, all 128
    output partitions used; softmax sum lands per-partition so
    normalization is reciprocal + per-partition scale on eviction.
    PSUM accumulation state is per-bank: each [128,65] group's 8
    matmuls stay contiguous, alternating between two banks.
  attn_outT[c,q] rebuilt with PE transposes (per head pair, per q-tile)
  y^T[co,t] = wproj^T @ attn_outT, + proj_b on eviction; fp16 out.
Head-pair production (QK), S, exp, O, transposes are interleaved
head-by-head (emission order = per-engine execution order) so PE /
ACT / DVE / Pool / DMA all stay busy; independent filler matmuls sit
between the exp-paced S chunks so the in-order PE never starves.
Host reassembles y from per-core y^T.
"""

import numpy as np

_CACHE = {}
_DEBUG = False

B = 8
WS = 32
N = WS * WS            # 1024 tokens
C = 768
NH = 12
HD = 64
P = 128
QC = 2                 # q chunks of 512
QN = N // QC           # 512
KT = N // P            # 8 k tiles
CT = C // P            # 6 contraction tiles
NJ = NH // 2           # 6 head pairs == Q (or K) out tiles
VC = 2                 # v output chunks of 384
VN = C // VC           # 384


def _build():
    import concourse.bass as bass
    import concourse.bacc as bacc
    import concourse.mybir as mybir
    import concourse.tile as tile

    f32 = mybir.dt.float32
    f16 = mybir.dt.float16
    AF = mybir.ActivationFunctionType

    nc = bacc.Bacc(None, target_bir_lowering=False)

    xT_d = nc.dram_tensor("xT", [C, N], f16, kind="ExternalInput")
    wqk_d = nc.dram_tensor("wqk", [C, NJ, 2 * P], f16, kind="ExternalInput")
    wv_d = nc.dram_tensor("wv", [C, C], f16, kind="ExternalInput")
    wproj_d = nc.dram_tensor("wproj", [C, C], f16, kind="ExternalInput")
    qb_d = nc.dram_tensor("qb", [NJ, P], f32, kind="ExternalInput")
    vb_d = nc.dram_tensor("vb", [C], f32, kind="ExternalInput")
    pb_d = nc.dram_tensor("pb", [CT, P], f32, kind="ExternalInput")
    biasT_d = nc.dram_tensor("biasT", [NH, N, N], f16, kind="ExternalInput")
    ident_d = nc.dram_tensor("ident", [P, P], f16, kind="ExternalInput")
    yT_d = nc.dram_tensor("yT", [C, N], f16, kind="ExternalOutput")
    if _DEBUG:
        dbg = {
            "d_qt": nc.dram_tensor("d_qt", [P, NJ, N], f16,
                                   kind="ExternalOutput"),
            "d_k2": nc.dram_tensor("d_k2", [P, NH, N], f16,
                                   kind="ExternalOutput"),
            "d_vaug": nc.dram_tensor("d_vaug", [P, KT, NH, HD + 1], f16,
                                     kind="ExternalOutput"),
            "d_ao": nc.dram_tensor("d_ao", [P, CT, N], f16,
                                   kind="ExternalOutput"),
            "d_pt0": nc.dram_tensor("d_pt0", [P, KT, N], f16,
                                    kind="ExternalOutput"),
            "d_pt1": nc.dram_tensor("d_pt1", [P, KT, N], f16,
                                    kind="ExternalOutput"),
            "d_on0": nc.dram_tensor("d_on0", [P, KT, 2, HD], f16,
                                    kind="ExternalOutput"),
            "d_po0": nc.dram_tensor("d_po0", [2, P, KT // 2, HD + 1], f32,
                                    kind="ExternalOutput"),
        }

    with tile.TileContext(nc) as tc:
        with (
            tc.tile_pool(name="cst", bufs=1) as cst,
            tc.tile_pool(name="bias_pool", bufs=3) as bias_pool,
            tc.tile_pool(name="small", bufs=8) as small,
            tc.tile_pool(name="pair_pool", bufs=2) as pair_pool,
            tc.tile_pool(name="pt_pool", bufs=2) as pt_pool,
            tc.tile_pool(name="yb_pool", bufs=3) as yb_pool,
        ):
            # ---- permanent buffers ----
            xT = cst.tile([P, CT, N], f16)
            wqk = cst.tile([P, CT, NJ, 2 * P], f16)
            wv = cst.tile([P, CT, C], f16)
            wproj = cst.tile([P, CT, C], f16)
            q_t = cst.tile([P, NJ, N], f16)        # Q^T packed per pair
            # K^T zero-padded per head: full-128-row stationary keeps FWL
            # enabled (64-row weights load measurably slower)
            k_pad = cst.tile([P, NH, N], f16)
            v_aug = cst.tile([P, KT, NH, HD + 1], f16)  # V + ones column
            attn_outT = cst.tile([P, CT, N], f16)
            ident = cst.tile([P, P], f16)
            qb = cst.tile([P, NJ], f32)
            vb_bc = cst.tile([P, C], f32)
            pbias = cst.tile([P, CT], f32)

            biasT = {}

            def load_bias(h):
                biasT[h] = bias_pool.tile([P, KT, N], f16, tag="biasT",
                                          name=f"biasT{h}")
                nc.sync.dma_start(
                    biasT[h], biasT_d[h].rearrange("(kt p) q -> p kt q", p=P))

            nc.sync.dma_start(qb, qb_d[:].rearrange("j p -> p j"))
            nc.sync.dma_start(ident, ident_d[:])
            nc.sync.dma_start(
                vb_bc, bass.AP(tensor=vb_d, offset=0, ap=[[0, P], [1, C]]))
            nc.sync.dma_start(pbias, pb_d[:].rearrange("j p -> p j"))
            nc.gpsimd.memset(v_aug[:, :, :, HD:HD + 1], 1.0)
            nc.gpsimd.memset(k_pad[HD:P, 0:NH:2, :], 0.0)
            nc.gpsimd.memset(k_pad[0:HD, 1:NH:2, :], 0.0)

            xT_src = xT_d[:].rearrange("(k p) t -> p k t", p=P)
            wv_src = wv_d[:].rearrange("(k p) o -> p k o", p=P)
            wqk_src = wqk_d[:].rearrange("(k p) j o -> p k j o", p=P)
            wproj_src = wproj_d[:].rearrange("(k p) o -> p k o", p=P)
            for k in range(CT):
                nc.sync.dma_start(xT[:, k, :], xT_src[:, k, :])
                nc.sync.dma_start(wv[:, k, :], wv_src[:, k, :])
            nc.sync.dma_start(wqk[:, :, 0, :], wqk_src[:, :, 0, :])
            nc.sync.dma_start(wqk[:, :, 1, :], wqk_src[:, :, 1, :])
            load_bias(0)
            nc.sync.dma_start(wqk[:, :, 2, :], wqk_src[:, :, 2, :])
            load_bias(1)
            for jj in range(3, NJ):
                nc.sync.dma_start(wqk[:, :, jj, :], wqk_src[:, :, jj, :])
            load_bias(2)
            for k in range(CT):
                nc.sync.dma_start(wproj[:, k, :], wproj_src[:, k, :])

            # ---- V projection ----
            # k-progressive in two tt-halves (8 psum banks, one group per
            # bank): first matmuls run as soon as the first xT/wv chunks
            # land instead of waiting for the full xT.
            with tc.tile_pool(name="ps_v", bufs=1, space="PSUM") as ps_v:
                for half in range(2):
                    tts = range(half * (KT // 2), (half + 1) * (KT // 2))
                    pvs = {(tt, vc): ps_v.tile(
                        [P, VN], f32, tag=f"pv{tt % 4}{vc}", name="pv")
                        for tt in tts for vc in range(VC)}
                    for k in range(CT):
                        for tt in tts:
                            for vc in range(VC):
                                nc.tensor.matmul(
                                    pvs[tt, vc],
                                    xT[:, k, tt * P:(tt + 1) * P],
                                    wv[:, k, vc * VN:(vc + 1) * VN],
                                    start=(k == 0), stop=(k == CT - 1))
                    for tt in tts:
                        for vc in range(VC):
                            h0 = vc * (NH // VC)
                            nc.vector.tensor_add(
                                v_aug[:, tt, h0:h0 + NH // VC, 0:HD],
                                pvs[tt, vc], vb_bc[:, vc * VN:(vc + 1) * VN])

            # ---- attention (QK pair production interleaved) ----
            # PE executes its queue in order, so independent "filler" work
            # (QK tiles for the next pair, AV matmuls for the previous head,
            # transposes) is emitted BETWEEN the exp-paced S chunks — the PE
            # then has runnable work during every pss-rotation stall.
            with (
                tc.tile_pool(name="ps_q", bufs=1, space="PSUM") as ps_q,
                tc.tile_pool(name="ps_s", bufs=2, space="PSUM") as ps_s,
                tc.tile_pool(name="ps_o", bufs=1, space="PSUM") as ps_o,
                tc.tile_pool(name="ps_t", bufs=1, space="PSUM") as ps_t,
            ):
                def qk_group(jj, i):
                    # one of 4 psum groups for pair jj: (Q|K) x q-chunk
                    which, qc = divmod(i, 2)
                    pq = ps_q.tile([P, QN], f32, tag="pq", name="pq")
                    for k in range(CT):
                        nc.tensor.matmul(
                            pq, wqk[:, k, jj, which * P:(which + 1) * P],
                            xT[:, k, qc * QN:(qc + 1) * QN],
                            start=(k == 0), stop=(k == CT - 1))
                    if which == 0:
                        nc.vector.tensor_scalar_add(
                            q_t[:, jj, qc * QN:(qc + 1) * QN], pq,
                            qb[:, jj:jj + 1])
                    else:
                        nc.vector.tensor_copy(
                            k_pad[0:HD, 2 * jj, qc * QN:(qc + 1) * QN],
                            pq[0:HD, :])
                        nc.vector.tensor_copy(
                            k_pad[HD:P, 2 * jj + 1, qc * QN:(qc + 1) * QN],
                            pq[HD:P, :])

                pt = {}
                o_nat = {}
                pos = {}

                def s_chunk(h, kt):
                    jj = h // 2
                    if kt == 0:
                        pt[h] = pt_pool.tile([P, KT, N], f16, tag="pt",
                                             name=f"pt{h}")
                    pss = ps_s.tile([P, N], f32, tag="pss", name="pss")
                    for qc in range(QC):
                        nc.tensor.matmul(
                            pss[:, qc * QN:(qc + 1) * QN],
                            k_pad[:, h, kt * P:(kt + 1) * P],
                            q_t[:, jj, qc * QN:(qc + 1) * QN],
                            start=True, stop=True)
                    nc.scalar.activation(
                        pt[h][:, kt, :], pss, AF.Exp, bias=0.0, scale=1.0)
                    eng = nc.gpsimd if kt % 3 == 0 else nc.vector
                    eng.tensor_mul(
                        pt[h][:, kt, :], pt[h][:, kt, :], biasT[h][:, kt, :])

                def o_pair(h, qi0):
                    # two accumulation groups interleaved across two PSUM
                    # banks: each bank still sees its group's 8 matmuls
                    # contiguously (accumulation state is per-bank), while
                    # alternating banks hides the accumulation-drain stall
                    # between consecutive same-bank matmuls
                    jj = h // 2
                    if qi0 == 0 and h % 2 == 0:
                        o_nat[jj] = pair_pool.tile(
                            [P, KT, 2, HD], f16, tag="onat", name=f"on{jj}")
                    poA = ps_o.tile([P, HD + 1], f32, tag="po0", name="po")
                    poB = ps_o.tile([P, HD + 1], f32, tag="po1", name="po")
                    for kt in range(KT):
                        nc.tensor.matmul(
                            poA, pt[h][:, kt, qi0 * P:(qi0 + 1) * P],
                            v_aug[:, kt, h, :],
                            start=(kt == 0), stop=(kt == KT - 1))
                        nc.tensor.matmul(
                            poB, pt[h][:, kt, (qi0 + 1) * P:(qi0 + 2) * P],
                            v_aug[:, kt, h, :],
                            start=(kt == 0), stop=(kt == KT - 1))
                    for qi, po in ((qi0, poA), (qi0 + 1, poB)):
                        if _DEBUG and h == 0:
                            stg = small.tile([P, HD + 1], f32, tag="postg",
                                             name="postg", bufs=2)
                            nc.vector.tensor_copy(stg, po)
                            nc.sync.dma_start(
                                dbg["d_po0"][qi // 4, :, qi % 4], stg)
                        inv = small.tile([P, 1], f32, tag="inv", name="inv")
                        nc.vector.reciprocal(inv, po[:, HD:HD + 1])
                        nc.vector.tensor_scalar_mul(
                            o_nat[jj][:, qi, h % 2, :], po[:, 0:HD], inv)

                def t_chunk(jj, qi):
                    tp = ps_t.tile([P, P], f16, tag="tp", name="tp")
                    nc.tensor.transpose(tp, o_nat[jj][:, qi, :, :], ident)
                    nc.vector.tensor_copy(
                        attn_outT[:, jj, qi * P:(qi + 1) * P], tp)

                for i in range(4):
                    qk_group(0, i)
                for h in range(NH):
                    if h + 3 < NH:
                        load_bias(h + 3)
                    for kt in range(KT):
                        s_chunk(h, kt)
                        # 2 QK psum-groups per head spread over heads 0-9:
                        # even head -> pair's Q halves, odd head -> K halves
                        if h // 2 + 1 < NJ and kt in (1, 5):
                            qk_group(h // 2 + 1,
                                     (2 if h % 2 else 0) + (kt == 5))
                        if h >= 3 and h % 2 == 1:
                            t_chunk(h // 2 - 1, kt)
                        if h >= 1 and kt % 2 == 1:
                            o_pair(h - 1, kt - 1)
                    if h >= 1:
                        del pt[h - 1]
                    if _DEBUG and h in (0, 1):
                        nc.sync.dma_start(dbg[f"d_pt{h}"][:], pt[h])
                    if _DEBUG and h == 2:
                        nc.sync.dma_start(dbg["d_on0"][:], o_nat[0])
                for qi0 in range(0, KT, 2):
                    o_pair(NH - 1, qi0)
                    if qi0 >= 2:
                        t_chunk(NJ - 1, qi0 - 2)
                        t_chunk(NJ - 1, qi0 - 1)
                del pt[NH - 1]
                t_chunk(NJ - 1, KT - 2)
                t_chunk(NJ - 1, KT - 1)
                if _DEBUG:
                    nc.sync.dma_start(dbg["d_qt"][:], q_t)
                    nc.sync.dma_start(dbg["d_k2"][:], k_pad)
                    nc.sync.dma_start(dbg["d_vaug"][:], v_aug)
                    nc.sync.dma_start(dbg["d_ao"][:], attn_outT)

            # ---- projection: y^T = wproj^T @ attn_outT ----
            with tc.tile_pool(name="ps_y", bufs=2, space="PSUM") as ps_y:
                for j in range(CT):
                    pys = [ps_y.tile([P, QN], f32, tag=f"py{qc}",
                                     name=f"py{qc}") for qc in range(QC)]
                    for k in range(CT):
                        for qc in range(QC):
                            nc.tensor.matmul(
                                pys[qc], wproj[:, k, j * P:(j + 1) * P],
                                attn_outT[:, k, qc * QN:(qc + 1) * QN],
                                start=(k == 0), stop=(k == CT - 1))
                    for qc in range(QC):
                        yb = yb_pool.tile([P, QN], f16, tag="yb", name="yb")
                        nc.scalar.activation(
                            yb, pys[qc], AF.Identity,
                            bias=pbias[:, j:j + 1], scale=1.0)
                        nc.sync.dma_start(
                            yT_d[:].rearrange("(j p) t -> p j t", p=P)
                            [:, j, qc * QN:(qc + 1) * QN], yb)

    nc.compile()
    return nc


def _get_nc():
    if "nc" not in _CACHE:
        _CACHE["nc"] = _build()
    return _CACHE["nc"]


def prepare_inputs(x, qkv_w, q_bias, v_bias, proj_w, proj_b, rel_table,
                   rel_index):
    """Host-side resharding/layout prep. Returns per-core input maps."""
    scale = HD ** -0.5
    x = np.asarray(x, np.float32)
    qkv_w = np.asarray(qkv_w, np.float32)
    q_bias = np.asarray(q_bias, np.float32)
    v_bias = np.asarray(v_bias, np.float32)
    proj_w = np.asarray(proj_w, np.float32)
    proj_b = np.asarray(proj_b, np.float32)
    rel_table = np.asarray(rel_table, np.float32)
    rel_index = np.asarray(rel_index)

    wq_t = (qkv_w[0:C, :] * scale).T           # [c, o] rows scaled
    wk_t = qkv_w[C:2 * C, :].T
    wv_t = np.ascontiguousarray(qkv_w[2 * C:3 * C, :].T.astype(np.float16))
    wproj_t = np.ascontiguousarray(proj_w.T.astype(np.float16))
    # pair-interleaved qk weights: [c, jj, (Q 128 | K 128)]
    wqk2 = np.empty((C, NJ, 2 * P), np.float32)
    for jj in range(NJ):
        wqk2[:, jj, 0:P] = wq_t[:, jj * P:(jj + 1) * P]
        wqk2[:, jj, P:2 * P] = wk_t[:, jj * P:(jj + 1) * P]
    wqk2 = np.ascontiguousarray(wqk2.astype(np.float16))
    qb = np.ascontiguousarray((q_bias * scale).reshape(NJ, P))
    pb = np.ascontiguousarray(proj_b.reshape(CT, P))

    # bias[q, k, h] = rel_table[rel_index[q, k]]; we ship exp(biasT[h, k, q])
    # so the kernel can fold the softmax bias multiplicatively into P^T
    bias = rel_table[rel_index.reshape(-1)].reshape(N, N, NH)
    biasT = np.ascontiguousarray(
        np.exp(bias.transpose(2, 1, 0), dtype=np.float32)).astype(np.float16)
    ident = np.eye(P, dtype=np.float16)

    shared = {
        "wqk": wqk2, "wv": wv_t, "wproj": wproj_t, "qb": qb,
        "vb": v_bias, "pb": pb, "biasT": biasT, "ident": ident,
    }
    in_maps = []
    for b in range(B):
        xt = np.ascontiguousarray(
            x[b].reshape(N, C).T.astype(np.float16))
        in_maps.append({"xT": xt, **shared})
    return in_maps


def kernel(x, qkv_w, q_bias, v_bias, proj_w, proj_b, rel_table, rel_index,
           _trace=False):
    from concourse.bass_utils import run_bass_kernel_spmd

    nc = _get_nc()
    in_maps = prepare_inputs(x, qkv_w, q_bias, v_bias, proj_w, proj_b,
                             rel_table, rel_index)
    kwargs = {}
    if _trace:
        import concourse.bass_utils as _bu
        _bu.upload_artifacts = lambda tmpdir: tmpdir
        kwargs = {"trace": True}
    res = run_bass_kernel_spmd(nc, in_maps, core_ids=list(range(B)), **kwargs)
    _CACHE["last_res"] = res
    out = np.empty((B, WS, WS, C), np.float32)
    for b in range(B):
        out[b] = res.results[b]["yT"].astype(np.float32).T.reshape(WS, WS, C)
    if _trace:
        _CACHE["last_result"] = res
    return out


# revision 41
# speedup vs baseline: 1.0798x; 1.0798x over previous
"""Trainium2 Bass kernel for windowed multi-head attention with relative
position bias (Swin-style block):

    qkv = x @ qkv_w.T + [q_bias, 0, v_bias]
    q, k, v = split(qkv);  q *= hd**-0.5
    attn = softmax(q @ k.T + rel_table[rel_index])
    out  = (attn @ v) @ proj_w.T + proj_b

Shapes: x [8, 32, 32, 768], 12 heads, head_dim 64, N=1024 tokens.

Sharding: pure data-parallel — one batch element per NeuronCore, 8 cores,
no collectives. Each core runs an identical NEFF on its own slice.

Per-core dataflow (fp16 operands, fp32 PSUM accumulate):
  xT [c,t] as lhsT, wv        -> V natural [t,o'] + ones col (v_aug)
  wqk pair-slices as lhsT, xT -> Q^T packed per pair / K^T zero-padded
    per head (full-128-row stationaries keep fast-weight-load enabled)
  S^T[k,q] = K_pad(h)^T @ Q  (per head, per k-tile; fp16)
  P^T = exp(S^T) (ACT) * expB (host-precomputed fp16 bias, DVE/Pool)
  O[q,d]+sums = pt-chunk^T @ [V_h|1]  -- natural orientation, all 128
    output partitions used; softmax sum lands per-partition so
    normalization is reciprocal + per-partition scale on eviction.
    PSUM accumulation state is per-bank: each [128,65] group's 8
    matmuls stay contiguous, alternating between two banks.
  attn_outT[c,q] rebuilt with PE transposes (per head pair, per q-tile)
  y^T[co,t] = wproj^T @ attn_outT, + proj_b on eviction; fp16 out.
Head-pair production (QK), S, exp, O, transposes are interleaved
head-by-head (emission order = per-engine execution order) so PE /
ACT / DVE / Pool / DMA all stay busy; independent filler matmuls sit
between the exp-paced S chunks so the in-order PE never starves.
Host reassembles y from per-core y^T.
"""

import numpy as np

_CACHE = {}
_DEBUG = False

B = 8
WS = 32
N = WS * WS            # 1024 tokens
C = 768
NH = 12
HD = 64
P = 128
QC = 2                 # q chunks of 512
QN = N // QC           # 512
KT = N // P            # 8 k tiles
CT = C // P            # 6 contraction tiles
NJ = NH // 2           # 6 head pairs == Q (or K) out tiles
VC = 2                 # v output chunks of 384
VN = C // VC           # 384


def _build():
    import concourse.bass as bass
    import concourse.bacc as bacc
    import concourse.mybir as mybir
    import concourse.tile as tile

    f32 = mybir.dt.float32
    f16 = mybir.dt.float16
    AF = mybir.ActivationFunctionType

    nc = bacc.Bacc(None, target_bir_lowering=False)

    xT_d = nc.dram_tensor("xT", [C, N], f16, kind="ExternalInput")
    wqk_d = nc.dram_tensor("wqk", [C, NJ, 2 * P], f16, kind="ExternalInput")
    wv_d = nc.dram_tensor("wv", [C, C], f16, kind="ExternalInput")
    wproj_d = nc.dram_tensor("wproj", [C, C], f16, kind="ExternalInput")
    qb_d = nc.dram_tensor("qb", [NJ, P], f32, kind="ExternalInput")
    vb_d = nc.dram_tensor("vb", [C], f32, kind="ExternalInput")
    pb_d = nc.dram_tensor("pb", [CT, P], f32, kind="ExternalInput")
    biasT_d = nc.dram_tensor("biasT", [NH, N, N], f16, kind="ExternalInput")
    ident_d = nc.dram_tensor("ident", [P, P], f16, kind="ExternalInput")
    yT_d = nc.dram_tensor("yT", [C, N], f16, kind="ExternalOutput")
    if _DEBUG:
        dbg = {
            "d_qt": nc.dram_tensor("d_qt", [P, NJ, N], f16,
                                   kind="ExternalOutput"),
            "d_k2": nc.dram_tensor("d_k2", [P, NH, N], f16,
                                   kind="ExternalOutput"),
            "d_vaug": nc.dram_tensor("d_vaug", [P, KT, NH, HD + 1], f16,
                                     kind="ExternalOutput"),
            "d_ao": nc.dram_tensor("d_ao", [P, CT, N], f16,
                                   kind="ExternalOutput"),
            "d_pt0": nc.dram_tensor("d_pt0", [P, KT, N], f16,
                                    kind="ExternalOutput"),
            "d_pt1": nc.dram_tensor("d_pt1", [P, KT, N], f16,
                                    kind="ExternalOutput"),
            "d_on0": nc.dram_tensor("d_on0", [P, KT, 2, HD], f16,
                                    kind="ExternalOutput"),
            "d_po0": nc.dram_tensor("d_po0", [2, P, KT // 2, HD + 1], f32,
                                    kind="ExternalOutput"),
        }

    with tile.TileContext(nc) as tc:
        with (
            tc.tile_pool(name="cst", bufs=1) as cst,
            tc.tile_pool(name="bias_pool", bufs=3) as bias_pool,
            tc.tile_pool(name="small", bufs=8) as small,
            tc.tile_pool(name="pair_pool", bufs=2) as pair_pool,
            tc.tile_pool(name="pt_pool", bufs=2) as pt_pool,
            tc.tile_pool(name="yb_pool", bufs=3) as yb_pool,
        ):
            # ---- permanent buffers ----
            xT = cst.tile([P, CT, N], f16)
            wqk = cst.tile([P, CT, NJ, 2 * P], f16)
            wv = cst.tile([P, CT, C], f16)
            wproj = cst.tile([P, CT, C], f16)
            q_t = cst.tile([P, NJ, N], f16)        # Q^T packed per pair
            # K^T zero-padded per head: full-128-row stationary keeps FWL
            # enabled (64-row weights load measurably slower)
            k_pad = cst.tile([P, NH, N], f16)
            v_aug = cst.tile([P, KT, NH, HD + 1], f16)  # V + ones column
            attn_outT = cst.tile([P, CT, N], f16)
            ident = cst.tile([P, P], f16)
            qb = cst.tile([P, NJ], f32)
            vb_bc = cst.tile([P, C], f32)
            pbias = cst.tile([P, CT], f32)

            biasT = {}

            def load_bias(h):
                biasT[h] = bias_pool.tile([P, KT, N], f16, tag="biasT",
                                          name=f"biasT{h}")
                nc.sync.dma_start(
                    biasT[h], biasT_d[h].rearrange("(kt p) q -> p kt q", p=P))

            nc.sync.dma_start(qb, qb_d[:].rearrange("j p -> p j"))
            nc.sync.dma_start(ident, ident_d[:])
            nc.sync.dma_start(
                vb_bc, bass.AP(tensor=vb_d, offset=0, ap=[[0, P], [1, C]]))
            nc.sync.dma_start(pbias, pb_d[:].rearrange("j p -> p j"))
            nc.gpsimd.memset(v_aug[:, :, :, HD:HD + 1], 1.0)
            nc.gpsimd.memset(k_pad[HD:P, 0:NH:2, :], 0.0)
            nc.gpsimd.memset(k_pad[0:HD, 1:NH:2, :], 0.0)

            xT_src = xT_d[:].rearrange("(k p) t -> p k t", p=P)
            wv_src = wv_d[:].rearrange("(k p) o -> p k o", p=P)
            wqk_src = wqk_d[:].rearrange("(k p) j o -> p k j o", p=P)
            wproj_src = wproj_d[:].rearrange("(k p) o -> p k o", p=P)
            for k in range(CT):
                nc.sync.dma_start(xT[:, k, :], xT_src[:, k, :])
                nc.sync.dma_start(wv[:, k, :], wv_src[:, k, :])
            nc.sync.dma_start(wqk[:, :, 0, :], wqk_src[:, :, 0, :])
            nc.sync.dma_start(wqk[:, :, 1, :], wqk_src[:, :, 1, :])
            load_bias(0)
            nc.sync.dma_start(wqk[:, :, 2, :], wqk_src[:, :, 2, :])
            load_bias(1)
            for jj in range(3, NJ):
                nc.sync.dma_start(wqk[:, :, jj, :], wqk_src[:, :, jj, :])
            load_bias(2)
            for k in range(CT):
                nc.sync.dma_start(wproj[:, k, :], wproj_src[:, k, :])

            # ps_q (1 bank) is usable DURING the V phase (V thirds use only
            # 6 banks) so pair-0 QK tiles overlap the DMA-paced V region
            # instead of serializing after it. ps_s/ps_o/ps_t banks are only
            # reserved from their first tile allocation, after ps_v closes.
            import contextlib
            with contextlib.ExitStack() as est:
                ps_q = est.enter_context(
                    tc.tile_pool(name="ps_q", bufs=1, space="PSUM"))

                def qk_group(jj, i):
                    # one of 4 psum groups for pair jj: (Q|K) x q-chunk
                    which, qc = divmod(i, 2)
                    pq = ps_q.tile([P, QN], f32, tag="pq", name="pq")
                    for k in range(CT):
                        nc.tensor.matmul(
                            pq, wqk[:, k, jj, which * P:(which + 1) * P],
                            xT[:, k, qc * QN:(qc + 1) * QN],
                            start=(k == 0), stop=(k == CT - 1))
                    if which == 0:
                        nc.vector.tensor_scalar_add(
                            q_t[:, jj, qc * QN:(qc + 1) * QN], pq,
                            qb[:, jj:jj + 1])
                    else:
                        nc.vector.tensor_copy(
                            k_pad[0:HD, 2 * jj, qc * QN:(qc + 1) * QN],
                            pq[0:HD, :])
                        nc.vector.tensor_copy(
                            k_pad[HD:P, 2 * jj + 1, qc * QN:(qc + 1) * QN],
                            pq[HD:P, :])

                # ---- V projection ----
                # k-progressive in tt-thirds (6 psum banks + ps_q's 1):
                # first matmuls run as soon as the first xT/wv chunks land,
                # and pair-0 QK tiles slot in before the last third.
                with tc.tile_pool(name="ps_v", bufs=1, space="PSUM") as ps_v:
                    def v_third(tts):
                        pvs = {(tt, vc): ps_v.tile(
                            [P, VN], f32, tag=f"pv{tt % 3}{vc}", name="pv")
                            for tt in tts for vc in range(VC)}
                        for k in range(CT):
                            for tt in tts:
                                for vc in range(VC):
                                    nc.tensor.matmul(
                                        pvs[tt, vc],
                                        xT[:, k, tt * P:(tt + 1) * P],
                                        wv[:, k, vc * VN:(vc + 1) * VN],
                                        start=(k == 0), stop=(k == CT - 1))
                        for tt in tts:
                            for vc in range(VC):
                                h0 = vc * (NH // VC)
                                nc.vector.tensor_add(
                                    v_aug[:, tt, h0:h0 + NH // VC, 0:HD],
                                    pvs[tt, vc],
                                    vb_bc[:, vc * VN:(vc + 1) * VN])

                    v_third((0, 1, 2))
                    v_third((3, 4, 5))
                    for i in range(4):
                        qk_group(0, i)
                    v_third((6, 7))

                # attention pools open only now — after ps_v closed — so
                # the allocator sees at most 8 concurrent PSUM banks
                ps_s = est.enter_context(
                    tc.tile_pool(name="ps_s", bufs=2, space="PSUM"))
                ps_o = est.enter_context(
                    tc.tile_pool(name="ps_o", bufs=1, space="PSUM"))
                ps_t = est.enter_context(
                    tc.tile_pool(name="ps_t", bufs=1, space="PSUM"))

                pt = {}
                o_nat = {}
                pos = {}

                def s_chunk(h, kt):
                    jj = h // 2
                    if kt == 0:
                        pt[h] = pt_pool.tile([P, KT, N], f16, tag="pt",
                                             name=f"pt{h}")
                    pss = ps_s.tile([P, N], f32, tag="pss", name="pss")
                    for qc in range(QC):
                        nc.tensor.matmul(
                            pss[:, qc * QN:(qc + 1) * QN],
                            k_pad[:, h, kt * P:(kt + 1) * P],
                            q_t[:, jj, qc * QN:(qc + 1) * QN],
                            start=True, stop=True)
                    nc.scalar.activation(
                        pt[h][:, kt, :], pss, AF.Exp, bias=0.0, scale=1.0)
                    eng = nc.gpsimd if kt % 3 == 0 else nc.vector
                    eng.tensor_mul(
                        pt[h][:, kt, :], pt[h][:, kt, :], biasT[h][:, kt, :])

                def o_pair(h, qi0):
                    # two accumulation groups interleaved across two PSUM
                    # banks: each bank still sees its group's 8 matmuls
                    # contiguously (accumulation state is per-bank), while
                    # alternating banks hides the accumulation-drain stall
                    # between consecutive same-bank matmuls
                    jj = h // 2
                    if qi0 == 0 and h % 2 == 0:
                        o_nat[jj] = pair_pool.tile(
                            [P, KT, 2, HD], f16, tag="onat", name=f"on{jj}")
                    poA = ps_o.tile([P, HD + 1], f32, tag="po0", name="po")
                    poB = ps_o.tile([P, HD + 1], f32, tag="po1", name="po")
                    for kt in range(KT):
                        nc.tensor.matmul(
                            poA, pt[h][:, kt, qi0 * P:(qi0 + 1) * P],
                            v_aug[:, kt, h, :],
                            start=(kt == 0), stop=(kt == KT - 1))
                        nc.tensor.matmul(
                            poB, pt[h][:, kt, (qi0 + 1) * P:(qi0 + 2) * P],
                            v_aug[:, kt, h, :],
                            start=(kt == 0), stop=(kt == KT - 1))
                    for qi, po in ((qi0, poA), (qi0 + 1, poB)):
                        if _DEBUG and h == 0:
                            stg = small.tile([P, HD + 1], f32, tag="postg",
                                             name="postg", bufs=2)
                            nc.vector.tensor_copy(stg, po)
                            nc.sync.dma_start(
                                dbg["d_po0"][qi // 4, :, qi % 4], stg)
                        inv = small.tile([P, 1], f32, tag="inv", name="inv")
                        nc.vector.reciprocal(inv, po[:, HD:HD + 1])
                        nc.vector.tensor_scalar_mul(
                            o_nat[jj][:, qi, h % 2, :], po[:, 0:HD], inv)

                def t_chunk(jj, qi):
                    tp = ps_t.tile([P, P], f16, tag="tp", name="tp")
                    nc.tensor.transpose(tp, o_nat[jj][:, qi, :, :], ident)
                    nc.vector.tensor_copy(
                        attn_outT[:, jj, qi * P:(qi + 1) * P], tp)

                for h in range(NH):
                    if h + 3 < NH:
                        load_bias(h + 3)
                    for kt in range(KT):
                        s_chunk(h, kt)
                        # 2 QK psum-groups per head spread over heads 0-9:
                        # even head -> pair's Q halves, odd head -> K halves
                        if h // 2 + 1 < NJ and kt in (1, 5):
                            qk_group(h // 2 + 1,
                                     (2 if h % 2 else 0) + (kt == 5))
                        if h >= 3 and h % 2 == 1:
                            t_chunk(h // 2 - 1, kt)
                        if h >= 1 and kt % 2 == 1:
                            o_pair(h - 1, kt - 1)
                    if h >= 1:
                        del pt[h - 1]
                    if _DEBUG and h in (0, 1):
                        nc.sync.dma_start(dbg[f"d_pt{h}"][:], pt[h])
                    if _DEBUG and h == 2:
                        nc.sync.dma_start(dbg["d_on0"][:], o_nat[0])
                for qi0 in range(0, KT, 2):
                    o_pair(NH - 1, qi0)
                    if qi0 >= 2:
                        t_chunk(NJ - 1, qi0 - 2)
                        t_chunk(NJ - 1, qi0 - 1)
                del pt[NH - 1]
                t_chunk(NJ - 1, KT - 2)
                t_chunk(NJ - 1, KT - 1)
                if _DEBUG:
                    nc.sync.dma_start(dbg["d_qt"][:], q_t)
                    nc.sync.dma_start(dbg["d_k2"][:], k_pad)
                    nc.sync.dma_start(dbg["d_vaug"][:], v_aug)
                    nc.sync.dma_start(dbg["d_ao"][:], attn_outT)

            # ---- projection: y^T = wproj^T @ attn_outT ----
            with tc.tile_pool(name="ps_y", bufs=2, space="PSUM") as ps_y:
                for j in range(CT):
                    pys = [ps_y.tile([P, QN], f32, tag=f"py{qc}",
                                     name=f"py{qc}") for qc in range(QC)]
                    for k in range(CT):
                        for qc in range(QC):
                            nc.tensor.matmul(
                                pys[qc], wproj[:, k, j * P:(j + 1) * P],
                                attn_outT[:, k, qc * QN:(qc + 1) * QN],
                                start=(k == 0), stop=(k == CT - 1))
                    for qc in range(QC):
                        yb = yb_pool.tile([P, QN], f16, tag="yb", name="yb")
                        nc.scalar.activation(
                            yb, pys[qc], AF.Identity,
                            bias=pbias[:, j:j + 1], scale=1.0)
                        nc.sync.dma_start(
                            yT_d[:].rearrange("(j p) t -> p j t", p=P)
                            [:, j, qc * QN:(qc + 1) * QN], yb)

    nc.compile()
    return nc


def _get_nc():
    if "nc" not in _CACHE:
        _CACHE["nc"] = _build()
    return _CACHE["nc"]


def prepare_inputs(x, qkv_w, q_bias, v_bias, proj_w, proj_b, rel_table,
                   rel_index):
    """Host-side resharding/layout prep. Returns per-core input maps."""
    scale = HD ** -0.5
    x = np.asarray(x, np.float32)
    qkv_w = np.asarray(qkv_w, np.float32)
    q_bias = np.asarray(q_bias, np.float32)
    v_bias = np.asarray(v_bias, np.float32)
    proj_w = np.asarray(proj_w, np.float32)
    proj_b = np.asarray(proj_b, np.float32)
    rel_table = np.asarray(rel_table, np.float32)
    rel_index = np.asarray(rel_index)

    wq_t = (qkv_w[0:C, :] * scale).T           # [c, o] rows scaled
    wk_t = qkv_w[C:2 * C, :].T
    wv_t = np.ascontiguousarray(qkv_w[2 * C:3 * C, :].T.astype(np.float16))
    wproj_t = np.ascontiguousarray(proj_w.T.astype(np.float16))
    # pair-interleaved qk weights: [c, jj, (Q 128 | K 128)]
    wqk2 = np.empty((C, NJ, 2 * P), np.float32)
    for jj in range(NJ):
        wqk2[:, jj, 0:P] = wq_t[:, jj * P:(jj + 1) * P]
        wqk2[:, jj, P:2 * P] = wk_t[:, jj * P:(jj + 1) * P]
    wqk2 = np.ascontiguousarray(wqk2.astype(np.float16))
    qb = np.ascontiguousarray((q_bias * scale).reshape(NJ, P))
    pb = np.ascontiguousarray(proj_b.reshape(CT, P))

    # bias[q, k, h] = rel_table[rel_index[q, k]]; we ship exp(biasT[h, k, q])
    # so the kernel can fold the softmax bias multiplicatively into P^T
    bias = rel_table[rel_index.reshape(-1)].reshape(N, N, NH)
    biasT = np.ascontiguousarray(
        np.exp(bias.transpose(2, 1, 0), dtype=np.float32)).astype(np.float16)
    ident = np.eye(P, dtype=np.float16)

    shared = {
        "wqk": wqk2, "wv": wv_t, "wproj": wproj_t, "qb": qb,
        "vb": v_bias, "pb": pb, "biasT": biasT, "ident": ident,
    }
    in_maps = []
    for b in range(B):
        xt = np.ascontiguousarray(
            x[b].reshape(N, C).T.astype(np.float16))
        in_maps.append({"xT": xt, **shared})
    return in_maps


def kernel(x, qkv_w, q_bias, v_bias, proj_w, proj_b, rel_table, rel_index,
           _trace=False):
    from concourse.bass_utils import run_bass_kernel_spmd

    nc = _get_nc()
    in_maps = prepare_inputs(x, qkv_w, q_bias, v_bias, proj_w, proj_b,
                             rel_table, rel_index)
    kwargs = {}
    if _trace:
        import concourse.bass_utils as _bu
        _bu.upload_artifacts = lambda tmpdir: tmpdir
        kwargs = {"trace": True}
    res = run_bass_kernel_spmd(nc, in_maps, core_ids=list(range(B)), **kwargs)
    _CACHE["last_res"] = res
    out = np.empty((B, WS, WS, C), np.float32)
    for b in range(B):
        out[b] = res.results[b]["yT"].astype(np.float32).T.reshape(WS, WS, C)
    if _trace:
        _CACHE["last_result"] = res
    return out
